# revision 10
# baseline (speedup 1.0000x reference)
"""Multi-head attention (B=1, S=4096, D=512, H=8, HD=64) on 8 trn2 NeuronCores.

Sharding: one head per core (tensor-parallel over heads). Each core computes
its head's Q/K/V projections, attention with a transposed flash-style layout
(scores^T = K Q^T with t on partitions; softmax denominator via a ones column
appended to V), applies its head's output projection, and writes a full
[S, D] partial. The host sums the 8 partials.

Pipeline: the whole kernel is one software pipeline. Projections (bf16
inputs) stream behind the x DMA during "chunk 0"; in steady state the PE
interleaves scores(c) with AV(c-1) (full-chunk decoupled through an e-tile
ring), exp runs on the Activation engine with a few groups per chunk
offloaded to the DVE via a Schraudolph bit-trick exp (int32 bit arithmetic),
and the y projection + writeback for chunk c-1 happens during chunk c.

Numerics: projections in bf16 (inputs are host-rounded), scores/AV/proj in
float32r, exp exact on Act and ~2%-per-element Schraudolph on DVE tiles
(softmax normalization cancels most of it; measured end-to-end rel err vs
the fp32 reference is a few e-3 against a 2e-2 gate).
"""

import numpy as np
import ml_dtypes

import concourse.bacc as bacc
import concourse.mybir as mybir
import concourse.tile as tile
from concourse.bass_utils import run_bass_kernel_spmd

S = 4096          # sequence length
D = 512           # model dim
HD = 64           # head dim
H = 8             # heads == cores
SCALE = HD ** -0.5
P = 128           # partitions
KT = D // P       # 4 k-tiles over the model dim
NSC = S // 512    # 8 s-chunks of 512
NTT = S // P      # 32 t-tiles of 128
NG = NTT // 2     # 16 groups of 2 t-tiles per chunk

F32 = mybir.dt.float32
F32R = mybir.dt.float32r
BF16 = mybir.dt.bfloat16
I16 = mybir.dt.int16

# Schraudolph magic (int32 / f32 bits). BIRSim's f32->i32 conversion is
# round-to-nearest-ish; SHIFT tuned for min end-to-end error.
LOG2E = float(np.log2(np.e))
SCH_A16 = float(SCALE * LOG2E * (1 << 7))


def r(ap):
    """fp32 AP -> float32r view (same bits, full-rate PE matmul)."""
    return ap.bitcast(F32R)


def build_kernel(dve_groups=(5, 10, 15), sch_shift=467232.0, warm_mms=3,
                 e_bufs=17, av7_lag=2):
    sch_b16 = float(127.0 * (1 << 7) - sch_shift / 65536.0)
    dve_groups = set(dve_groups)
    Exp = mybir.ActivationFunctionType.Exp

    nc = bacc.Bacc(
        "TRN2",
        target_bir_lowering=False,
        debug=False,
        enable_asserts=False,
        num_devices=H,
    )

    xt = nc.dram_tensor("xt", [D, S], BF16, kind="ExternalInput").ap()
    wqk = nc.dram_tensor("wqk", [D, P], BF16, kind="ExternalInput").ap()
    wv = nc.dram_tensor("wv", [D, HD], BF16, kind="ExternalInput").ap()
    wp = nc.dram_tensor("wp", [HD, D], F32, kind="ExternalInput").ap()
    y = nc.dram_tensor("y", [S, D], F32, kind="ExternalOutput").ap()

    with tile.TileContext(nc) as tc:
        with (
            tc.tile_pool(name="const", bufs=1) as cp,
            tc.tile_pool(name="epool", bufs=e_bufs) as ep,
            tc.tile_pool(name="spsum", bufs=2, space="PSUM") as sp,
            tc.tile_pool(name="ys", bufs=4) as ysp,
        ):
            # ---- persistent SBUF ----
            xt_sb = cp.tile([P, KT, S], BF16, tag="xt")
            wqk_sb = cp.tile([P, KT, P], BF16, tag="wqk")
            wv_sb = cp.tile([P, KT, HD], BF16, tag="wv")
            wp_sb = cp.tile([HD, D], F32, tag="wp")
            # Q^T on rows 0-63 (psum copy) and rows 64-127 (dup DMA);
            # K^T on rows 64-127 (psum copy). Scores run on partitions 64-127.
            qq = cp.tile([P, S], F32, tag="qq")
            kk = cp.tile([P, S], F32, tag="kk")
            v_sb = cp.tile([P, NTT, HD + 1], F32, tag="v")
            v_bf = cp.tile([P, NTT, HD + 1], BF16, tag="vbf")
            o_sb = cp.tile([HD + 1, S], F32, tag="o")
            rz = cp.tile([P, NSC * KT], F32, tag="rz")          # 1/Z
            ones_sb = cp.tile([HD + 1, 1], F32, tag="ones")
            warm = cp.tile([P, D], BF16, tag="warm")
            ones_pre = cp.tile([P, NTT, 1], F32, tag="ones_pre")

            # ---- const / weight loads (Act queue; they gate QK(0)) ----
            nc.scalar.dma_start(wqk_sb, wqk.rearrange("(a p) d -> p a d", p=P))
            nc.scalar.dma_start(wv_sb, wv.rearrange("(a p) d -> p a d", p=P))
            nc.scalar.dma_start(r(wp_sb), r(wp))
            nc.vector.memset(warm, 0.25)
            nc.vector.memset(ones_pre, 1.0)
            nc.vector.tensor_copy(r(v_sb[:, :, HD : HD + 1]), ones_pre)
            nc.vector.tensor_copy(v_bf[:, :, HD : HD + 1], ones_pre)
            nc.vector.memset(ones_sb, 1.0)

            # ---- x load: alternate SP / Act queues ----
            xt_r = xt.rearrange("(a p) s -> p a s", p=P)
            for c in range(NSC):
                ssl = slice(c * 512, (c + 1) * 512)
                nc.sync.dma_start(xt_sb[:, :, ssl], xt_r[:, :, ssl])

            def exp_group(c, g, s_ps):
                """exp of score group (c, g): [128t, 1024] psum -> e tile.

                Act groups: exact exp, f32r out. DVE groups: Schraudolph
                bit-trick exp producing bf16 bits through an int16 view; the
                matching AV matmuls then run bf16 x bf16.
                """
                if g in dve_groups:
                    e_t = ep.tile([P, 1024], BF16, tag="e16", bufs=7)
                    nc.vector.tensor_scalar(
                        e_t.bitcast(I16), s_ps, SCH_A16, sch_b16,
                        mybir.AluOpType.mult, mybir.AluOpType.add,
                    )
                else:
                    e_t = ep.tile([P, 1024], F32, tag="e", bufs=14)
                    nc.scalar.activation(r(e_t), s_ps, Exp, scale=SCALE)
                return e_t

            def scores_group(c, g):
                s_ps = sp.tile([P, 1024], F32, tag="s")
                for i in range(2):
                    t = 2 * g + i
                    nc.tensor.matmul(
                        s_ps[:, i * 512 : (i + 1) * 512],
                        r(kk[HD:, t * P : (t + 1) * P]),
                        r(qq[HD:, c * 512 : (c + 1) * 512]),
                        start=True, stop=True,
                    )
                return s_ps

            def av_group(c, g, o_ps, e_tile):
                bf = g in dve_groups
                for i in range(2):
                    t = 2 * g + i
                    v_ap = v_bf[:, t, :] if bf else r(v_sb[:, t, :])
                    e_ap = e_tile[:, i * 512 : (i + 1) * 512]
                    nc.tensor.matmul(
                        o_ps, v_ap, e_ap if bf else r(e_ap),
                        start=(g == 0 and i == 0), stop=(g == NG - 1 and i == 1),
                    )

            def end_of_chunk(c):
                """o copy + Z transpose (K=1 matmuls) + reciprocal, chunk c."""
                ssl = slice(c * 512, (c + 1) * 512)
                zsl = slice(c * KT, (c + 1) * KT)
                nc.vector.tensor_copy(r(o_sb[:, ssl]), o_chunk[c])
                for j in range(KT):
                    st = c * KT + j
                    nc.tensor.matmul(
                        zt_ps[:, st : st + 1],
                        o_sb[HD : HD + 1, st * P : (st + 1) * P],
                        ones_sb[HD : HD + 1, :],
                        start=True, stop=True,
                    )
                nc.vector.reciprocal(rz[:, zsl], zt_ps[:, zsl])

            def y_tile(st):
                """project + scale + write s-tile st."""
                stsl = slice(st * P, (st + 1) * P)
                y_ps = yp.tile([P, D], F32, tag="y")
                nc.tensor.matmul(y_ps, r(o_sb[:HD, stsl]), r(wp_sb),
                                 start=True, stop=True)
                ys_t = ysp.tile([P, D], F32, tag="ys")
                nc.vector.tensor_scalar(
                    ys_t, y_ps, rz[:, st : st + 1], None, mybir.AluOpType.mult
                )
                nc.sync.dma_start(y[stsl, :], ys_t)

            e_ring = {}     # (c, g) -> e tile
            o_chunk = {}    # c -> o_ps tile

            # ---- chunk 0: warmup + projections + scores(0) ----
            with tc.tile_pool(name="projpsum", bufs=2, space="PSUM") as pp:
                wm_ps = pp.tile([P, D], F32, tag="proj")
                for i in range(warm_mms):
                    nc.tensor.matmul(wm_ps, warm[:, :P], warm,
                                     start=True, stop=True, skip_group_check=True)
                for c in range(NSC):
                    csl = slice(c * 512, (c + 1) * 512)
                    # QK packed projection for s-chunk c
                    qk_ps = pp.tile([P, D], F32, tag="proj")
                    for a in range(KT):
                        nc.tensor.matmul(qk_ps, wqk_sb[:, a, :], xt_sb[:, a, csl],
                                         start=(a == 0), stop=(a == KT - 1))
                    nc.vector.tensor_copy(r(qq[:HD, csl]), qk_ps[:HD])
                    nc.vector.tensor_copy(r(kk[HD:, csl]), qk_ps[HD:])
                    nc.scalar.dma_start(r(qq[HD:, csl]), r(qq[:HD, csl]))
                    # V projection for t-tiles 4c..4c+3 (natural layout)
                    v_ps = pp.tile([P, D], F32, tag="proj")
                    for j in range(KT):
                        t = KT * c + j
                        tsl = slice(t * P, (t + 1) * P)
                        for a in range(KT):
                            nc.tensor.matmul(
                                v_ps[:, j * HD : (j + 1) * HD],
                                xt_sb[:, a, tsl], wv_sb[:, a, :],
                                start=(a == 0), stop=(a == KT - 1),
                            )
                    v_quad = v_ps[:, : KT * HD].rearrange("p (j d) -> p j d", j=KT)
                    nc.vector.tensor_copy(r(v_sb[:, KT * c : KT * c + KT, :HD]), v_quad)
                    nc.vector.tensor_copy(v_bf[:, KT * c : KT * c + KT, :HD], v_quad)
                    # scores for groups 2c, 2c+1 of chunk 0
                    for g in (2 * c, 2 * c + 1):
                        s_ps = scores_group(0, g)
                        e_ring[(0, g)] = exp_group(0, g, s_ps)

            # ---- steady chunks 1..7 ----
            with (
                tc.tile_pool(name="opsum", bufs=2, space="PSUM") as op,
                tc.tile_pool(name="ypsum", bufs=1, space="PSUM") as yp,
                tc.tile_pool(name="ztpsum", bufs=1, space="PSUM") as ztp,
            ):
                zt_ps = ztp.tile([P, NSC * KT], F32, tag="zt")
                for c in range(1, NSC):
                    o_chunk[c - 1] = op.tile([HD + 1, 512], F32, tag="o", name="o_ps")
                    for g in range(NG):
                        s_ps = scores_group(c, g)
                        av_group(c - 1, g, o_chunk[c - 1], e_ring.pop((c - 1, g)))
                        e_ring[(c, g)] = exp_group(c, g, s_ps)
                        # trailing AV(7) inside chunk 7
                        if c == NSC - 1 and g >= av7_lag:
                            if g == av7_lag:
                                o_chunk[c] = op.tile([HD + 1, 512], F32, tag="o", name="o_ps")
                            av_group(c, g - av7_lag, o_chunk[c],
                                     e_ring.pop((c, g - av7_lag)))
                        # y for chunk c-2 spread through chunk c
                        if c >= 2 and g in (4, 7, 10, 13):
                            y_tile((c - 2) * KT + {4: 0, 7: 1, 10: 2, 13: 3}[g])
                    end_of_chunk(c - 1)

                # ---- tail: AV(7) remainder, then y(6), y(7) ----
                c = NSC - 1
                for g in range(NG - av7_lag, NG):
                    av_group(c, g, o_chunk[c], e_ring.pop((c, g)))
                end_of_chunk(c)
                for st in range((NSC - 2) * KT, NSC * KT):
                    y_tile(st)

    nc.compile()
    return nc


def run(inputs, trace=False, **build_kwargs):
    x = np.asarray(inputs["x"], dtype=np.float32)
    q_param = np.asarray(inputs["q_param"], dtype=np.float32)
    k_param = np.asarray(inputs["k_param"], dtype=np.float32)
    v_param = np.asarray(inputs["v_param"], dtype=np.float32)
    p_param = np.asarray(inputs["p_param"], dtype=np.float32)

    xt = np.ascontiguousarray(x[0].T).astype(ml_dtypes.bfloat16)  # [D, S]
    in_maps = []
    for h in range(H):
        wqk = np.concatenate([q_param[:, h, :], k_param[:, h, :]], axis=1)
        in_maps.append(
            {
                "xt": xt,
                "wqk": np.ascontiguousarray(wqk).astype(ml_dtypes.bfloat16),
                "wv": np.ascontiguousarray(v_param[:, h, :]).astype(ml_dtypes.bfloat16),
                "wp": np.ascontiguousarray(p_param[h]),
            }
        )

    nc = build_kernel(**build_kwargs)
    res = run_bass_kernel_spmd(nc, in_maps, core_ids=list(range(H)), trace=trace)
    out = np.zeros((S, D), dtype=np.float32)
    for h in range(H):
        out += res.results[h]["y"]
    return out[None, :, :], res


def kernel(**inputs) -> np.ndarray:
    out, _ = run(inputs, trace=False)
    return out
